# revision 1
# baseline (speedup 1.0000x reference)
"""ImprovedGRUCell Trainium2 kernel (8-core data-parallel over batch).

Layout strategy:
  - batch sharded 8 ways (8192 rows/core); 256x256 weights replicated.
  - Per core, batch super-tiles of 512 rows. Activations x/h loaded natural
    ([128 part = batch, free = features]), cast to bf16, transposed to
    feature-major ([128 part = feature, free = batch]) via DMA xbar
    transpose.  All matmuls contract features on the partition dim:
      * z / h-candidate branches computed in natural orientation
        (psum [batch, hidden]) -> softmax-free ops are per-partition.
      * attention branch computed transposed (psum [hidden, batch]) so the
        v_a scale and exp are per-partition; softmax denominator is taken
        back to natural orientation with tiny N=1 matmuls so the reciprocal
        and normalize are per-partition scalars.
  - sigmoid(s) avoided (different ACT table set than exp): z = 0.5*tanh(s/2)+0.5
    folded into the final blend arithmetic. ACT only uses {Tanh, Exp, Copy}.
"""

import os
import sys

sys.path.insert(0, "/opt/trn_rl_repo")

import ml_dtypes
import numpy as np

import concourse.bass as bass
import concourse.mybir as mybir
from concourse import bacc, tile
from concourse.bass_utils import run_bass_kernel_spmd

B_TOTAL = 65536
N_CORES = 8
B_CORE = B_TOTAL // N_CORES  # 8192
D = 256
ST = 512  # batch rows per super-tile
N_ST = B_CORE // ST

F32 = mybir.dt.float32
BF16 = mybir.dt.bfloat16
AF = mybir.ActivationFunctionType
ALU = mybir.AluOpType

_CACHE = {}

WNAMES = ("wzt", "uzt", "wat", "uat", "wht", "uht")


def build_nc(use_bias=False):
    nc = bacc.Bacc(
        "TRN2",
        target_bir_lowering=False,
        debug=False,
        enable_asserts=False,
        num_devices=N_CORES,
    )

    x_d = nc.dram_tensor("x", [B_CORE, D], F32, kind="ExternalInput")
    h_d = nc.dram_tensor("h", [B_CORE, D], F32, kind="ExternalInput")
    w_d = {
        n: nc.dram_tensor(n, [D, D], BF16, kind="ExternalInput") for n in WNAMES
    }
    bz_d = nc.dram_tensor("bz", [1, D], BF16, kind="ExternalInput")
    bh_d = nc.dram_tensor("bh", [1, D], BF16, kind="ExternalInput")
    va_d = nc.dram_tensor("va", [D], F32, kind="ExternalInput")
    id_d = nc.dram_tensor("ident", [128, 128], BF16, kind="ExternalInput")
    out_d = nc.dram_tensor("out", [B_CORE, D], F32, kind="ExternalOutput")

    xpose = os.environ.get("XPOSE", "pe")  # pe | xbar3d | xbar2d

    with tile.TileContext(nc) as tc:
        with (
            tc.tile_pool(name="wpool", bufs=1) as wp,
            tc.tile_pool(name="io", bufs=3) as io,
            tc.tile_pool(name="wk", bufs=2) as wk,
            tc.tile_pool(name="psn", bufs=3, space="PSUM") as psn,
            tc.tile_pool(name="psa", bufs=2, space="PSUM") as psa,
            tc.tile_pool(name="psd", bufs=1, space="PSUM") as psd,
            tc.tile_pool(name="pst", bufs=2, space="PSUM") as pst,
        ):
            # ---- persistent weights -------------------------------------
            w_sb = {}
            for n in WNAMES:
                t = wp.tile([128, 2 * D], BF16, tag=n)
                nc.sync.dma_start(
                    out=t.rearrange("p (kt h) -> p kt h", kt=2),
                    in_=w_d[n].ap().rearrange("(kt p) h -> p kt h", p=128),
                )
                w_sb[n] = t
            bz_sb = wp.tile([1, D], BF16, tag="bz")
            nc.sync.dma_start(out=bz_sb[:], in_=bz_d.ap())
            bh_sb = wp.tile([1, D], BF16, tag="bh")
            nc.sync.dma_start(out=bh_sb[:], in_=bh_d.ap())
            va_sb = wp.tile([128, 2], F32, tag="va")
            nc.sync.dma_start(
                out=va_sb[:], in_=va_d.ap().rearrange("(t p) -> p t", p=128)
            )
            ones_r = wp.tile([1, 128], BF16, tag="ones_r")  # K=1 lhsT for bias
            nc.vector.memset(ones_r[:], 1.0)
            ones_c = wp.tile([128, 1], BF16, tag="ones_c")  # rhs for denom
            nc.vector.memset(ones_c[:], 1.0)
            ident = wp.tile([128, 128], BF16, tag="ident")
            nc.sync.dma_start(out=ident[:], in_=id_d.ap())

            # ---- main loop over batch super-tiles -----------------------
            for st in range(N_ST):
                b0 = st * ST
                xn = io.tile([128, 4 * D], F32, tag="xn")
                hn = io.tile([128, 4 * D], F32, tag="hn")
                nc.sync.dma_start(
                    out=xn.rearrange("p (j k) -> p j k", j=4),
                    in_=x_d.ap()[b0 : b0 + ST, :].rearrange("(j p) k -> p j k", p=128),
                )
                nc.sync.dma_start(
                    out=hn.rearrange("p (j k) -> p j k", j=4),
                    in_=h_d.ap()[b0 : b0 + ST, :].rearrange("(j p) k -> p j k", p=128),
                )

                xb = wk.tile([128, 4 * D], BF16, tag="xb")
                nc.vector.tensor_copy(xb[:], xn[:])
                hb = wk.tile([128, 4 * D], BF16, tag="hb")
                nc.vector.tensor_copy(hb[:], hn[:])

                # transposed (feature-major) copies: t[kk, (j*2+kt)*128 + p]
                xT = wk.tile([128, 4 * D], BF16, tag="xT")
                hT = wk.tile([128, 4 * D], BF16, tag="hT")
                if xpose == "xbar2d":
                    for src, dst in ((xb, xT), (hb, hT)):
                        for j in range(4):
                            for kt in range(2):
                                e = (j * 2 + kt) * 128
                                nc.sync.dma_start_transpose(
                                    out=dst[:, e : e + 128],
                                    in_=src[:, j * D + kt * 128 : j * D + (kt + 1) * 128],
                                )
                elif xpose == "xbar3d":
                    nc.sync.dma_start_transpose(
                        out=xT.rearrange("k (e b) -> k e b", e=8), in_=xb[:]
                    )
                    nc.sync.dma_start_transpose(
                        out=hT.rearrange("k (e b) -> k e b", e=8), in_=hb[:]
                    )
                else:  # PE-transpose via identity matmul, psum staged in bf16
                    for src, dst in ((xb, xT), (hb, hT)):
                        dst4 = dst.rearrange("k (j kt b) -> k j kt b", j=4, kt=2)
                        for kt in range(2):
                            pt = pst.tile([128, 512], BF16, tag="pst")
                            pt3 = pt.rearrange("k (j b) -> k j b", j=4)
                            for j in range(4):
                                nc.tensor.transpose(
                                    pt3[:, j],
                                    src[:, j * D + kt * 128 : j * D + (kt + 1) * 128],
                                    ident[:],
                                )
                            nc.scalar.activation(dst4[:, :, kt, :], pt3, AF.Copy)

                xT4 = xT.rearrange("k (j kt b) -> k j kt b", j=4, kt=2)
                hT4 = hT.rearrange("k (j kt b) -> k j kt b", j=4, kt=2)

                # ---- z branch (natural): t_z = tanh(S_z / 2) ------------
                tz = wk.tile([128, 4 * D], F32, tag="tz")
                for jp in range(2):
                    pz = psn.tile([128, 512], F32, tag="psn")
                    for jj in range(2):
                        j = jp * 2 + jj
                        sl = slice(jj * D, (jj + 1) * D)
                        for kt in range(2):
                            nc.tensor.matmul(
                                pz[:, sl],
                                xT[:, (j * 2 + kt) * 128 : (j * 2 + kt + 1) * 128],
                                w_sb["wzt"][:, kt * D : (kt + 1) * D],
                                start=(kt == 0),
                                stop=False,
                            )
                        for kt in range(2):
                            nc.tensor.matmul(
                                pz[:, sl],
                                hT[:, (j * 2 + kt) * 128 : (j * 2 + kt + 1) * 128],
                                w_sb["uzt"][:, kt * D : (kt + 1) * D],
                                start=False,
                                stop=(not use_bias and kt == 1),
                            )
                        if use_bias:
                            nc.tensor.matmul(
                                pz[:, sl], ones_r[:], bz_sb[:], start=False, stop=True
                            )
                    nc.scalar.activation(
                        tz[:, jp * 512 : (jp + 1) * 512], pz[:], AF.Tanh, scale=0.5
                    )

                # ---- attention branch (transposed) ----------------------
                A_sb = wk.tile([128, 4 * D], BF16, tag="A")  # tanh(S_a)^T
                for ht in range(2):
                    pa = psa.tile([128, 512], F32, tag="psa")
                    pa3 = pa.rearrange("h (j b) -> h j b", j=4)
                    for kt in range(2):
                        nc.tensor.matmul(
                            pa3,
                            w_sb["wat"][:, kt * D + ht * 128 : kt * D + ht * 128 + 128],
                            xT4[:, :, kt, :],
                            start=(kt == 0),
                            stop=False,
                        )
                    for kt in range(2):
                        nc.tensor.matmul(
                            pa3,
                            w_sb["uat"][:, kt * D + ht * 128 : kt * D + ht * 128 + 128],
                            hT4[:, :, kt, :],
                            start=False,
                            stop=(kt == 1),
                        )
                    nc.scalar.activation(
                        A_sb[:, ht * 512 : (ht + 1) * 512], pa[:], AF.Tanh
                    )

                E_sb = wk.tile([128, 4 * D], BF16, tag="E")  # exp(va*A)^T
                for ht in range(2):
                    nc.scalar.activation(
                        E_sb[:, ht * 512 : (ht + 1) * 512],
                        A_sb[:, ht * 512 : (ht + 1) * 512],
                        AF.Exp,
                        scale=va_sb[:, ht : ht + 1],
                    )

                # softmax denominators, natural orientation [128 batch, 4]
                pd = psd.tile([128, 4], F32, tag="psd")
                for j in range(4):
                    for ht in range(2):
                        nc.tensor.matmul(
                            pd[:, j : j + 1],
                            E_sb[:, ht * 512 + j * 128 : ht * 512 + (j + 1) * 128],
                            ones_c[:],
                            start=(ht == 0),
                            stop=(ht == 1),
                        )
                r_sb = wk.tile([128, 4], F32, tag="r")
                nc.vector.reciprocal(r_sb[:], pd[:])

                # unnormalized attended_h^T = E^T * h^T  (bf16, e-layout)
                att = wk.tile([128, 4 * D], BF16, tag="att")
                att4 = att.rearrange("k (j kt b) -> k j kt b", j=4, kt=2)
                E4 = E_sb.rearrange("k (t j b) -> k t j b", t=2, j=4)
                for ht in range(2):
                    nc.vector.tensor_mul(
                        att4[:, :, ht, :], E4[:, ht], hT4[:, :, ht, :]
                    )

                # ---- candidate branch (natural) -------------------------
                sW = wk.tile([128, 4 * D], F32, tag="sW")
                Sh = wk.tile([128, 4 * D], F32, tag="Sh")
                for jp in range(2):
                    pw = psn.tile([128, 512], F32, tag="psn")
                    for jj in range(2):
                        j = jp * 2 + jj
                        sl = slice(jj * D, (jj + 1) * D)
                        for kt in range(2):
                            nc.tensor.matmul(
                                pw[:, sl],
                                xT[:, (j * 2 + kt) * 128 : (j * 2 + kt + 1) * 128],
                                w_sb["wht"][:, kt * D : (kt + 1) * D],
                                start=(kt == 0),
                                stop=(not use_bias and kt == 1),
                            )
                        if use_bias:
                            nc.tensor.matmul(
                                pw[:, sl], ones_r[:], bh_sb[:], start=False, stop=True
                            )
                    nc.scalar.activation(
                        sW[:, jp * 512 : (jp + 1) * 512], pw[:], AF.Copy
                    )
                    pu = psn.tile([128, 512], F32, tag="psn")
                    for jj in range(2):
                        j = jp * 2 + jj
                        sl = slice(jj * D, (jj + 1) * D)
                        for ht in range(2):
                            nc.tensor.matmul(
                                pu[:, sl],
                                att[:, (j * 2 + ht) * 128 : (j * 2 + ht + 1) * 128],
                                w_sb["uht"][:, ht * D : (ht + 1) * D],
                                start=(ht == 0),
                                stop=(ht == 1),
                            )
                    for jj in range(2):
                        j = jp * 2 + jj
                        sl = slice(jj * D, (jj + 1) * D)
                        slg = slice(j * D, (j + 1) * D)
                        nc.vector.scalar_tensor_tensor(
                            Sh[:, slg],
                            pu[:, sl],
                            r_sb[:, j : j + 1],
                            sW[:, slg],
                            op0=ALU.mult,
                            op1=ALU.add,
                        )
                htl = wk.tile([128, 4 * D], F32, tag="htl")
                nc.scalar.activation(htl[:], Sh[:], AF.Tanh)

                # ---- blend: h_t = h + (0.5*t_z+0.5)*(h~ - h) ------------
                zb = wk.tile([128, 4 * D], F32, tag="zbl")
                nc.vector.tensor_scalar(
                    zb[:], tz[:], 0.5, 0.5, op0=ALU.mult, op1=ALU.add
                )
                dbl = wk.tile([128, 4 * D], F32, tag="dbl")
                nc.gpsimd.tensor_sub(dbl[:], htl[:], hn[:])
                qbl = wk.tile([128, 4 * D], F32, tag="qbl")
                nc.gpsimd.tensor_mul(qbl[:], zb[:], dbl[:])
                ot = io.tile([128, 4 * D], F32, tag="ot")
                nc.vector.tensor_add(ot[:], qbl[:], hn[:])
                nc.sync.dma_start(
                    out=out_d.ap()[b0 : b0 + ST, :].rearrange(
                        "(j p) k -> p j k", p=128
                    ),
                    in_=ot.rearrange("p (j k) -> p j k", j=4),
                )

    nc.compile()
    return nc


LAST_RESULTS = None


def kernel(x, h_prev, W_z, U_z, b_z, W_a, U_a, v_a, W_h, U_h, b_h):
    global LAST_RESULTS
    use_bias = bool(np.any(np.asarray(b_z)) or np.any(np.asarray(b_h)))
    key = ("nc", use_bias)
    if key not in _CACHE:
        _CACHE[key] = build_nc(use_bias)
    nc = _CACHE[key]

    bf = ml_dtypes.bfloat16
    x = np.ascontiguousarray(np.asarray(x, dtype=np.float32))
    h_prev = np.ascontiguousarray(np.asarray(h_prev, dtype=np.float32))
    wmats = {
        "wzt": W_z,
        "uzt": U_z,
        "wat": W_a,
        "uat": U_a,
        "wht": W_h,
        "uht": U_h,
    }
    common = {
        n: np.ascontiguousarray(np.asarray(m, dtype=np.float32).T.astype(bf))
        for n, m in wmats.items()
    }
    common["bz"] = np.asarray(b_z, dtype=np.float32).reshape(1, D).astype(bf)
    common["bh"] = np.asarray(b_h, dtype=np.float32).reshape(1, D).astype(bf)
    common["va"] = np.ascontiguousarray(np.asarray(v_a, dtype=np.float32))
    common["ident"] = np.eye(128, dtype=bf)

    in_maps = []
    for c in range(N_CORES):
        m = dict(common)
        m["x"] = x[c * B_CORE : (c + 1) * B_CORE]
        m["h"] = h_prev[c * B_CORE : (c + 1) * B_CORE]
        in_maps.append(m)

    LAST_RESULTS = run_bass_kernel_spmd(nc, in_maps, core_ids=list(range(N_CORES)))
    outs = LAST_RESULTS.results
    return np.concatenate([outs[c]["out"] for c in range(N_CORES)], axis=0)

